# revision 12
# baseline (speedup 1.0000x reference)
"""Trainium2 Bass kernel for nn_Mixture_Loss_74053826118054.

Strategy (data parallel over batch B=256, 32 batches per core):
  Every term of the loss depends only on 5 per-(s,b)-row reductions over D:
    ll = sum_d l^2,  tt = sum_d t^2,  lt = sum_d l*t,
    ln = sum_d l[s]*l[s+1]  (consecutive sentences, same batch),
    tn = sum_d t[s]*t[s+1]
  (masked MSE = sum over valid rows of ll - 2lt + tt; cosines = dots/norms).

  Layout: rows are batch-major (b, s). Each SBUF partition holds a window of
  9 consecutive rows (8 + 1 overlap), so consecutive-row products are
  free-axis slices — no partition shifts (illegal on compute engines).
  l and t are stacked into one DRAM tensor and each 1024-column chunk is
  fetched with a single DMA (so every compute op needs at most one new
  semaphore: TPB instructions only encode one sync wait). Per chunk j:
  ACT computes both squares with fused accumulate, GpSimd computes l*t with
  fused accumulate, DVE computes the two shifted products with fused
  tensor_tensor_reduce. Fused-op full-size `out` operands are tiny (128,1)
  dummies broadcast to shape; each op gets a unique dummy so no WAW sems
  appear. The overlap row costs +12.5% DMA on a memory-bound kernel.

  The tiny O(S*B) finish (cos, deltas, compaction, delta-of-delta) runs on
  host in float64.
"""

import numpy as np

import concourse.bass as bass
import concourse.mybir as mybir
from concourse import tile
from concourse.bass_utils import run_bass_kernel_spmd

F32 = mybir.dt.float32
AF = mybir.ActivationFunctionType
ALU = mybir.AluOpType

N_CORES = 8
S, B, D = 64, 256, 1024
B_SHARD = B // N_CORES          # 32 batches per core
ROWS = B_SHARD * S              # 2048 real rows per core
G = 16                          # rows per window
P = 128                         # partitions per tile
NMEGA = ROWS // (G * P)         # 2 window-sets per core
ROWS_PAD = (P * NMEGA + 1) * G  # 2056: one extra window of padding rows
NCOL = NMEGA * G                # 16 result columns
QUANTS = ("ll", "tt", "lt", "ln", "tn")

_cached_nc = None


def _build_program():
    global _cached_nc
    if _cached_nc is not None:
        return _cached_nc

    nc = bass.Bass()
    x_in = nc.dram_tensor("x", [2, ROWS_PAD, D], F32, kind="ExternalInput")
    outs = {q: nc.dram_tensor(q, [P, NCOL], F32, kind="ExternalOutput")
            for q in QUANTS}

    # (half, rows, d) -> (window, half, slot, d) so a (128, 2, 1024) chunk is
    # one strided DMA: partition = window, middle dim = l/t half.
    x_v = x_in.rearrange("h (w g) d -> w h g d", g=G)   # (129, 2, 16, 1024)

    CW = 2 * D                       # chunk width in the big tile
    Q = 4                            # chunks per batched op group

    with tile.TileContext(nc) as tc:
        with tc.tile_pool(name="inp", bufs=1) as inp, \
             tc.tile_pool(name="scr", bufs=1) as scr, \
             tc.tile_pool(name="res", bufs=1) as res:
            rt = {q: res.tile([P, NCOL], F32, tag=q, name=f"rt_{q}")
                  for q in QUANTS}

            # whole per-core input resident in SBUF: chunk j at column j*CW,
            # l half at +0, t half at +D
            xbig = inp.tile([P, (G + 1) * CW], F32, name="xbig")
            for j in range(G + 1):
                if j < G:
                    nc.sync.dma_start(out=xbig[:, j * CW:(j + 1) * CW],
                                      in_=x_v[0:P, :, j, :])
                else:  # overlap row: next window's slot 0
                    nc.sync.dma_start(out=xbig[:, j * CW:(j + 1) * CW],
                                      in_=x_v[1:P + 1, :, 0, :])

            xq = xbig[:].rearrange("p (c d) -> p c d", d=D)  # (128, 34, 1024)

            def quad(base_slot):   # (128, Q, 1024) strided view
                return xq[:, base_slot:base_slot + 2 * Q - 1:2, :]

            for qi in range(G // Q):
                j0 = qi * Q
                col = j0
                l_q = quad(2 * j0)          # l halves of chunks j0..j0+3
                t_q = quad(2 * j0 + 1)      # t halves
                ln_q = quad(2 * j0 + 2)     # l halves of chunks j0+1..j0+4
                tn_q = quad(2 * j0 + 3)

                pl = scr.tile([P, Q, D], F32, tag="pl", name=f"pl_{qi}")
                pt = scr.tile([P, Q, D], F32, tag="pt", bufs=2, name=f"pt_{qi}")
                pc = scr.tile([P, Q, D], F32, tag="pc", name=f"pc_{qi}")

                # products
                nc.vector.tensor_tensor(out=pl[:], in0=l_q, in1=ln_q,
                                        op=ALU.mult)
                nc.gpsimd.tensor_tensor(out=pt[:], in0=t_q, in1=tn_q,
                                        op=ALU.mult)
                nc.gpsimd.tensor_tensor(out=pc[:], in0=l_q, in1=t_q,
                                        op=ALU.mult)

                # reduces: ln + lt quad-reduced on DVE, tn per-chunk on ACT
                nc.vector.tensor_reduce(
                    out=rt["ln"][:, col:col + Q].rearrange("p (c u) -> p c u",
                                                           u=1),
                    in_=pl[:], op=ALU.add, axis=mybir.AxisListType.X)
                nc.vector.tensor_reduce(
                    out=rt["lt"][:, col:col + Q].rearrange("p (c u) -> p c u",
                                                           u=1),
                    in_=pc[:], op=ALU.add, axis=mybir.AxisListType.X)

                def dummy(kind, k):
                    return scr.tile([P, 1], F32, tag=f"{kind}{qi}_{k}",
                                    name=f"{kind}_{qi}_{k}")

                for k in range(Q):
                    c = col + k
                    nc.scalar.activation(
                        dummy("da", k).broadcast_to((P, D)),
                        xq[:, 2 * (j0 + k), :], AF.Square,
                        accum_out=rt["ll"][:, c:c + 1])
                    nc.scalar.activation(
                        dummy("db", k).broadcast_to((P, D)),
                        xq[:, 2 * (j0 + k) + 1, :], AF.Square,
                        accum_out=rt["tt"][:, c:c + 1])
                    nc.scalar.activation(
                        dummy("dc", k).broadcast_to((P, D)),
                        pt[:, k, :], AF.Copy,
                        accum_out=rt["tn"][:, c:c + 1])

            for q in QUANTS:
                nc.sync.dma_start(out=outs[q][:], in_=rt[q][:])

    _legalize_waits(nc)
    _cached_nc = nc
    return nc


def _legalize_waits(nc):
    """Walrus encodes at most one sync wait per TPB instruction. Split any
    non-DMA instruction carrying N>1 waits into N-1 preceding same-engine
    EventSemaphore waits plus the instruction keeping one wait."""
    dummy_sem = nc.alloc_semaphore("legalize_pad")
    cur_insts = nc.cur_bb.bb.instructions
    for bb in nc.main_func.blocks:
        insts = bb.instructions
        new_list = []
        changed = False
        for ins in insts:
            si = ins.sync_info
            waits = list(si.on_wait) if si is not None and si.on_wait else []
            if len(waits) > 1:
                for w in waits[:-1]:
                    ev = nc.engines[ins.engine].wait_ge(dummy_sem, 0).ins
                    # wait_ge appends to the current block; reclaim it
                    popped = cur_insts.pop()
                    assert popped is ev
                    ev.sync_info.on_wait = [w]
                    new_list.append(ev)
                si.on_wait = [waits[-1]]
                changed = True
            new_list.append(ins)
        if changed:
            insts[:] = new_list


def _unpack(arr):
    """(128, NCOL) device layout -> (B_SHARD, S): row r = i*1024 + p*8 + j."""
    return (arr.reshape(P, NMEGA, G).transpose(1, 0, 2)
            .reshape(ROWS).reshape(B_SHARD, S)) if NMEGA > 1 else \
        arr.reshape(ROWS).reshape(B_SHARD, S)


def _run_device(logits, tgt_out, trace=False):
    """Returns dict q -> (B, S) float32 row-dot arrays, plus kernel results."""
    nc = _build_program()
    # (S, B, D) -> (B, S, D) batch-major, split over cores along B
    lb = np.ascontiguousarray(np.swapaxes(logits, 0, 1))
    tb = np.ascontiguousarray(np.swapaxes(tgt_out, 0, 1))
    in_maps = []
    for c in range(N_CORES):
        sl = slice(c * B_SHARD, (c + 1) * B_SHARD)
        x = np.zeros((2, ROWS_PAD, D), np.float32)
        x[0, :ROWS] = lb[sl].reshape(ROWS, D)
        x[1, :ROWS] = tb[sl].reshape(ROWS, D)
        in_maps.append({"x": x})
    kres = run_bass_kernel_spmd(nc, in_maps, list(range(N_CORES)), trace=trace)
    full = {}
    for q in QUANTS:
        full[q] = np.concatenate(
            [_unpack(kres.results[c][q]) for c in range(N_CORES)], axis=0)
    return full, kres


def _finish_host(rows, mask):
    """Host-side float64 finish: reproduce reference semantics exactly."""
    ll = rows["ll"].astype(np.float64)
    tt = rows["tt"].astype(np.float64)
    lt = rows["lt"].astype(np.float64)
    ln = rows["ln"].astype(np.float64)
    tn = rows["tn"].astype(np.float64)

    valid = ~mask                     # (B, S)
    n_valid = float(valid.sum())

    # masked MSE: sum over valid rows of sum_d (l-t)^2 = ll - 2lt + tt
    mse = ((ll - 2.0 * lt + tt) * valid).sum() / (n_valid * D)

    # CosineEmbeddingLoss part (eps = 1e-8)
    na = np.maximum(np.sqrt(ll), 1e-8)
    nb = np.maximum(np.sqrt(tt), 1e-8)
    c = lt / (na * nb)
    loss_cos = ((1.0 - c) * valid).sum() / n_valid

    # consecutive-sentence cosine deltas (eps = 1e-6), shape (B, S-1)
    nl = np.maximum(np.sqrt(ll), 1e-6)
    nt = np.maximum(np.sqrt(tt), 1e-6)
    d_l = ln[:, :S - 1] / (nl[:, :-1] * nl[:, 1:])
    d_t = tn[:, :S - 1] / (nt[:, :-1] * nt[:, 1:])
    pair_valid = valid[:, :-1] & valid[:, 1:]
    cnt = int(pair_valid.sum())
    loss_delta = (np.square(d_l - d_t) * pair_valid).sum() / max(cnt, 1)

    # delta-of-delta on the compacted (valid-only, batch-major) delta lists
    L = B * (S - 1)
    pvf = pair_valid.reshape(-1)

    def dd(d_flat):
        dense = np.zeros(L, np.float64)
        dense[:cnt] = d_flat[pvf]
        prev = dense[:-1]
        den = np.where(prev != 0, prev, 1e-6)
        return (dense[1:] - prev) / den

    dd_l = dd(d_l.reshape(-1))
    dd_t = dd(d_t.reshape(-1))
    dd_valid = np.arange(L - 1) < (cnt - 1)
    n_dd = float(max(cnt - 1, 1))
    loss_dd = (np.square(dd_l - dd_t) * dd_valid).sum() / n_dd / 100.0

    return mse + loss_cos + loss_delta + loss_dd


def kernel(logits, tgt_out, tgt_padding_mask, _trace=False):
    logits = np.asarray(logits, dtype=np.float32)
    tgt_out = np.asarray(tgt_out, dtype=np.float32)
    mask = np.asarray(tgt_padding_mask).astype(bool)
    rows, kres = _run_device(logits, tgt_out, trace=_trace)
    total = _finish_host(rows, mask)
    out = np.array(total, dtype=np.float32)
    if _trace:
        return out, kres
    return out
